# revision 4
# baseline (speedup 1.0000x reference)
"""Multi-head attention (B=2, S=2048, D=1024, H=16) on 8 NeuronCores.

Sharding: core c -> (batch b = c // 4, head-group g = c % 4, 4 heads each).
Each core computes its 4 heads' attention for its batch plus the partial
output projection (ctx_shard @ WO_shard.T).T; the host sums the 4 partials
per batch, adds the bias, and patches fully-masked query rows (where the
reference's softmax degenerates to uniform attention).

Device kernel layout notes:
  - x and the weight shards are pre-transposed on the host and fed as bf16.
  - Q,K are produced in [dk, s] layout (head-pair stacked on partitions) so
    scores come out transposed: S_t[k, q]. The two heads of a pair run as
    concurrent row-group matmuls (K=64 each).
  - Padding mask is applied by zeroing masked rows of V (and of the ones
    column), causal mask by multiplying the 128-col diagonal block of band
    tiles with a precomputed 0/1 triangle.
  - Softmax normalization is deferred past the attention loop: V carries an
    extra ones column so P@V also accumulates row sums L[q]; unnormalized
    ctx and L are staged to SBUF (bf16), L rows bounce through DRAM where a
    reshape to [128, f] makes the reciprocal cheap, and a partition-broadcast
    DMA returns 1/L for the normalize multiplies.
  - Emission order sets scheduler priority: attention(0,0) is emitted right
    after its minimal projection prefix so exp starts early; the remaining
    projections and the per-qt out-projections are emitted later and the
    Tile list scheduler hoists them into the exp-paced PE gaps.
  - A burst of dummy matmuls at t=0 warms the PE HAM clock gate during the
    input-DMA window.
"""

import os
import sys

import numpy as np

sys.path.insert(0, "/opt/trn_rl_repo")
os.environ.setdefault("MYCRO_LOCAL_CACHE", "1")

import ml_dtypes

import concourse.bass as bass
import concourse.tile as tile
from concourse import bacc, mybir
from concourse.bass_utils import run_bass_kernel_spmd

B, S, D, H = 2, 2048, 1024, 16
DK = D // H          # 64
NCORES = 8
HPC = H // (NCORES // B)   # heads per core = 4
DSH = HPC * DK             # 256: per-core shard of the model dim
NKC = S // 128             # 16 key chunks of 128
TRI_W = 384 + 512          # causal strip width
N_WARM = 120               # HAM warmup matmuls

BF = mybir.dt.bfloat16
F32 = mybir.dt.float32
EXP = mybir.ActivationFunctionType.Exp

_NC_CACHE: list = []


def _emit(tc: tile.TileContext, ctx):
    nc = tc.nc

    xT = nc.dram_tensor("xT", [D, S], BF, kind="ExternalInput").ap()
    wqt = nc.dram_tensor("wqt", [D, DSH], BF, kind="ExternalInput").ap()
    wkt = nc.dram_tensor("wkt", [D, DSH], BF, kind="ExternalInput").ap()
    wvt = nc.dram_tensor("wvt", [D, DSH], BF, kind="ExternalInput").ap()
    wot = nc.dram_tensor("wot", [DSH, D], BF, kind="ExternalInput").ap()
    pad0 = nc.dram_tensor("pad0", [128, NKC], F32, kind="ExternalInput").ap()
    tri = nc.dram_tensor("tri", [128, TRI_W], BF, kind="ExternalInput").ap()
    yT = nc.dram_tensor("yT", [D, S], BF, kind="ExternalOutput").ap()

    persist = ctx.enter_context(tc.tile_pool(name="persist", bufs=1))
    sc_pool = ctx.enter_context(tc.tile_pool(name="scps", bufs=2, space="PSUM"))
    ct_pool = ctx.enter_context(tc.tile_pool(name="ctps", bufs=4, space="PSUM"))
    pu_pool = ctx.enter_context(tc.tile_pool(name="pu", bufs=4))
    work = ctx.enter_context(tc.tile_pool(name="work", bufs=4))
    dpool = ctx.enter_context(tc.tile_pool(name="dram", bufs=1, space="DRAM"))

    xs = persist.tile([128, 8, S], BF)
    wq_s = persist.tile([128, 8, DSH], BF)
    wk_s = persist.tile([128, 8, DSH], BF)
    wv_s = persist.tile([128, 8, DSH], BF)
    wo_s = persist.tile([128, 2, D], BF)
    pad_s = persist.tile([128, NKC], F32)
    tri_s = persist.tile([128, TRI_W], BF)
    qt2 = persist.tile([128, 2, S], BF)
    kt2 = persist.tile([128, 2, S], BF)
    vp = persist.tile([128, NKC, 65 * HPC], BF)
    ctn = persist.tile([128, 2, S], BF)
    ctu = persist.tile([65, 16, 512], BF)     # unnormalized ctx + L, per (h, qt)
    wrm = persist.tile([128, 128], BF)        # HAM warmup operand
    rc0 = persist.tile([128, 32], BF)         # pair-0 L reshaped for reciprocal
    rc1 = [persist.tile([128, 8], BF, name=f"rc1_{i}", tag=f"rc1_{i}")
           for i in range(4)]
    ldram = dpool.tile([16, 512], BF)         # raw L rows, r = hp*8 + qt*2 + idx
    ldram2 = dpool.tile([16, 512], BF)        # 1/L rows

    # ---- HAM warmup: keep the PE busy through the input-DMA window ----
    nc.vector.memset(wrm, 0.0)
    wps = ct_pool.tile([64, 64], F32, tag="ct", name="wps")
    for _ in range(N_WARM):
        nc.tensor.matmul(wps, wrm[:, 0:64], wrm[:, 64:128], start=True, stop=True)

    # ---- input DMAs; scalar queue kept clean so EXP dispatch is not delayed
    xr = xT.rearrange("(c p) s -> p c s", p=128)
    wqr = wqt.rearrange("(c p) j -> p c j", p=128)
    wkr = wkt.rearrange("(c p) j -> p c j", p=128)
    wvr = wvt.rearrange("(c p) j -> p c j", p=128)
    nc.sync.dma_start(out=pad_s, in_=pad0)
    nc.sync.dma_start(out=tri_s, in_=tri)
    engs = [nc.sync, nc.gpsimd]
    ei = 0

    def dma_rr(out, in_):
        nonlocal ei
        engs[ei % len(engs)].dma_start(out=out, in_=in_)
        ei += 1

    for c in range(8):
        dma_rr(wq_s[:, c, :], wqr[:, c, :])
        dma_rr(wk_s[:, c, :], wkr[:, c, :])
        dma_rr(xs[:, c, 0:1024], xr[:, c, 0:1024])
        dma_rr(wv_s[:, c, :], wvr[:, c, :])
    for c in range(8):
        dma_rr(xs[:, c, 1024:2048], xr[:, c, 1024:2048])
    wor = wot.rearrange("(c p) o -> p c o", p=128)
    for c in range(2):
        dma_rr(wo_s[:, c, :], wor[:, c, :])
    nc.vector.memset(vp, 1.0)

    def qk_proj(dht, use_act, sts):
        """Project Q and K s-tiles `sts` for head pair `dht`."""
        for wi, (wsb, dst) in enumerate(((wq_s, qt2), (wk_s, kt2))):
            pss = [
                ct_pool.tile([128, 512], F32, tag="ct", name=f"qkps{si}")
                for si in range(len(sts))
            ]
            for dc in range(8):
                for st, ps in zip(sts, pss):
                    nc.tensor.matmul(
                        ps,
                        wsb[:, dc, 128 * dht : 128 * dht + 128],
                        xs[:, dc, 512 * st : 512 * st + 512],
                        start=(dc == 0),
                        stop=(dc == 7),
                    )
            for st, ps in zip(sts, pss):
                sl = dst[:, dht, 512 * st : 512 * st + 512]
                if use_act:
                    nc.scalar.copy(sl, ps)
                else:
                    nc.vector.tensor_copy(out=sl, in_=ps)

    def v_proj(sc_lo, sc_hi):
        for sc in range(sc_lo, sc_hi):
            ps = ct_pool.tile([128, DSH], F32, tag="ct")
            for dc in range(8):
                nc.tensor.matmul(
                    ps,
                    xs[:, dc, 128 * sc : 128 * sc + 128],
                    wv_s[:, dc, :],
                    start=(dc == 0),
                    stop=(dc == 7),
                )
            vcols = vp[:, sc, :].rearrange("p (h u) -> p h u", u=65)[:, :, 0:64]
            nc.vector.tensor_scalar_mul(
                vcols, ps.rearrange("p (h u) -> p h u", u=64), pad_s[:, sc : sc + 1]
            )
            ones_cols = vp[:, sc, :].rearrange("p (h u) -> p h u", u=65)[:, :, 64:65]
            nc.vector.tensor_scalar_mul(ones_cols, ones_cols, pad_s[:, sc : sc + 1])

    def attention(hp, qt):
        Q0 = 512 * qt
        nkc = 4 * qt + 4
        ct_e = ct_pool.tile([65, 512], F32, tag="ct")
        ct_o = ct_pool.tile([65, 512], F32, tag="ct")
        he, ho = 2 * hp, 2 * hp + 1
        for g in range(0, nkc, 2):
            group = []
            # scores for both kc of the group first: lets exp(kc) overlap
            # scores(kc+1) and batches the K=64 row-group config on the PE
            for kc in (g, g + 1):
                K0 = 128 * kc
                band = K0 >= Q0
                qs = K0 if band else Q0
                w = Q0 + 512 - qs
                co = qs - Q0
                sc = sc_pool.tile([128, 1024], F32, tag="slot")
                nc.tensor.matmul(
                    sc[:, 0:w], kt2[0:64, hp, K0 : K0 + 128],
                    qt2[0:64, hp, qs : qs + w], start=True, stop=True,
                )
                nc.tensor.matmul(
                    sc[:, 512 : 512 + w], kt2[64:128, hp, K0 : K0 + 128],
                    qt2[64:128, hp, qs : qs + w], start=True, stop=True,
                )
                group.append((kc, band, w, co, sc))
            pus = []
            for kc, band, w, co, sc in group:
                pu = pu_pool.tile([128, 1024], BF, tag="pu")
                sc2 = sc.rearrange("p (t f) -> p t f", t=2)[:, :, 0:w]
                pu2 = pu.rearrange("p (t f) -> p t f", t=2)[:, :, 0:w]
                nc.scalar.activation(out=pu2, in_=sc2, func=EXP, scale=0.125)
                if band:
                    # only the leading 128 cols hold the diagonal triangle;
                    # the rest of the band tile is fully live
                    mw = min(128, w)
                    tsl = tri_s[:, 384 : 384 + mw]
                    nc.vector.tensor_mul(pu[:, 0:mw], pu[:, 0:mw], tsl)
                    nc.vector.tensor_mul(
                        pu[:, 512 : 512 + mw], pu[:, 512 : 512 + mw], tsl
                    )
                pus.append(pu)
            for (kc, band, w, co, sc), pu in zip(group, pus):
                nc.tensor.matmul(
                    ct_e[:, co : co + w],
                    vp[:, kc, 65 * he : 65 * he + 65], pu[:, 0:w],
                    start=(kc == 0), stop=(kc == nkc - 1),
                )
                nc.tensor.matmul(
                    ct_o[:, co : co + w],
                    vp[:, kc, 65 * ho : 65 * ho + 65], pu[:, 512 : 512 + w],
                    start=(kc == 0), stop=(kc == nkc - 1),
                )
        for idx, cta in ((0, ct_e), (1, ct_o)):
            hq = (2 * hp + idx) * 4 + qt
            nc.vector.tensor_copy(out=ctu[:, hq, :], in_=cta)
            r = hp * 8 + qt * 2 + idx
            nc.sync.dma_start(out=ldram[r : r + 1, :], in_=ctu[64:65, hq, :])

    def _bcast64(r):
        """1/L row r of ldram2 -> [64, 512] SBUF via partition-broadcast DMA."""
        rlb = work.tile([64, 512], BF, tag="rlb")
        src_row = ldram2[r : r + 1, :]
        bsrc = bass.AP(
            tensor=src_row.tensor, offset=src_row.offset,
            ap=[[0, 64]] + list(src_row.ap[1:]),
        )
        nc.sync.dma_start(out=rlb, in_=bsrc)
        return rlb

    def _recip(dst, lo_r, n_r):
        """1/L for ldram rows [lo_r, lo_r+n_r) via [128, f] reshape."""
        f = n_r * 512 // 128
        src = ldram[lo_r : lo_r + n_r, :].rearrange("r (q j) -> (r q) j", j=f)
        nc.gpsimd.dma_start(out=dst, in_=src)
        nc.vector.tensor_scalar_max(dst, dst, 1e-30)
        with nc.allow_low_precision(reason="1/L in bf16; rel-err budget is 2e-2"):
            nc.vector.reciprocal(dst, dst)
        out = ldram2[lo_r : lo_r + n_r, :].rearrange("r (q j) -> (r q) j", j=f)
        nc.gpsimd.dma_start(out=out, in_=dst)

    def norm_apply(hp, qt, idx):
        Q0 = 512 * qt
        hq = (2 * hp + idx) * 4 + qt
        rlb = _bcast64(hp * 8 + qt * 2 + idx)
        if idx == 0:
            nc.vector.tensor_mul(
                ctn[0:64, hp, Q0 : Q0 + 512], ctu[0:64, hq, :], rlb
            )
        else:
            stg = work.tile([64, 512], BF, tag="stg")
            nc.vector.tensor_mul(stg, ctu[0:64, hq, :], rlb)
            nc.gpsimd.dma_start(out=ctn[64:128, hp, Q0 : Q0 + 512], in_=stg)

    def norm_pair0():
        _recip(rc0, 0, 8)
        for qt in range(4):
            for idx in (0, 1):
                norm_apply(0, qt, idx)

    def norm_qt1(qt):
        _recip(rc1[qt], 8 + 2 * qt, 2)
        for idx in (0, 1):
            norm_apply(1, qt, idx)

    def out_proj(st, tail=False):
        yr = yT.rearrange("(ot p) s -> ot p s", p=128)
        for ot in range(8):
            ps = ct_pool.tile([128, 512], F32, tag="ct")
            for c2 in range(2):
                nc.tensor.matmul(
                    ps,
                    wo_s[:, c2, 128 * ot : 128 * ot + 128],
                    ctn[:, c2, 512 * st : 512 * st + 512],
                    start=(c2 == 0),
                    stop=(c2 == 1),
                )
            ystg = work.tile([128, 512], BF, tag="y")
            if tail and ot % 2 == 0:
                nc.scalar.copy(ystg, ps)
            else:
                nc.vector.tensor_copy(out=ystg, in_=ps)
            nc.gpsimd.dma_start(out=yr[ot, :, 512 * st : 512 * st + 512], in_=ystg)

    # ---- emission order == scheduler priority ----
    qk_proj(0, use_act=True, sts=(0,))
    v_proj(0, 4)
    attention(0, 0)
    qk_proj(0, use_act=True, sts=(1,))
    v_proj(4, 8)
    attention(0, 1)
    qk_proj(0, use_act=False, sts=(2,))
    v_proj(8, 12)
    attention(0, 2)
    qk_proj(0, use_act=False, sts=(3,))
    v_proj(12, 16)
    attention(0, 3)
    qk_proj(1, use_act=False, sts=(0, 1))
    norm_pair0()
    attention(1, 0)
    qk_proj(1, use_act=False, sts=(2, 3))
    norm_qt1(0)
    attention(1, 1)
    out_proj(0)
    norm_qt1(1)
    attention(1, 2)
    out_proj(1)
    norm_qt1(2)
    attention(1, 3)
    out_proj(2)
    norm_qt1(3)
    out_proj(3, tail=True)


def build_nc():
    nc = bacc.Bacc(
        "TRN2",
        target_bir_lowering=False,
        debug=False,
        enable_asserts=False,
        num_devices=NCORES,
    )
    from contextlib import ExitStack

    with tile.TileContext(nc) as tc:
        with ExitStack() as ctx:
            _emit(tc, ctx)
    nc.compile()
    return nc


def _get_nc():
    if not _NC_CACHE:
        _NC_CACHE.append(build_nc())
    return _NC_CACHE[0]


def make_tri() -> np.ndarray:
    p = np.arange(128)[:, None]
    v = np.arange(TRI_W)[None, :]
    return (p <= v - 384).astype(np.float32).astype(ml_dtypes.bfloat16)


def make_in_maps(x, mask, WQ, WK, WV, WO):
    bf = ml_dtypes.bfloat16
    tri = make_tri()
    in_maps = []
    for c in range(NCORES):
        b, g = c // (NCORES // B), c % (NCORES // B)
        sl = slice(DSH * g, DSH * g + DSH)
        in_maps.append(
            {
                "xT": np.ascontiguousarray(x[b].T).astype(bf),
                "wqt": np.ascontiguousarray(WQ[sl, :].T).astype(bf),
                "wkt": np.ascontiguousarray(WK[sl, :].T).astype(bf),
                "wvt": np.ascontiguousarray(WV[sl, :].T).astype(bf),
                "wot": np.ascontiguousarray(WO[:, sl].T).astype(bf),
                "pad0": np.ascontiguousarray(
                    (mask[b] == 0).astype(np.float32).reshape(NKC, 128).T
                ),
                "tri": tri,
            }
        )
    return in_maps


def assemble(results, x, mask, WV, WO, bO) -> np.ndarray:
    y = np.zeros((B, S, D), np.float32)
    for c in range(NCORES):
        y[c // (NCORES // B)] += results[c]["yT"].T
    y += bO[None, None, :]
    # Rows i < first-unmasked-index are fully masked in the reference; its
    # softmax then degenerates to uniform attention over all positions.
    for b in range(B):
        nz = np.nonzero(mask[b] == 0)[0]
        t = int(nz[0]) if nz.size else S
        if t > 0:
            vbar = x[b].mean(axis=0) @ WV.T
            yfix = vbar @ WO.T + bO
            y[b, :t, :] = yfix
    return y


def kernel(x, mask, WQ, WK, WV, WO, bO) -> np.ndarray:
    x = np.asarray(x, np.float32)
    mask = np.asarray(mask, np.int32)
    WQ = np.asarray(WQ, np.float32)
    WK = np.asarray(WK, np.float32)
    WV = np.asarray(WV, np.float32)
    WO = np.asarray(WO, np.float32)
    bO = np.asarray(bO, np.float32)

    nc = _get_nc()
    in_maps = make_in_maps(x, mask, WQ, WK, WV, WO)
    res = run_bass_kernel_spmd(nc, in_maps, list(range(NCORES)))
    return assemble(res.results, x, mask, WV, WO, bO)


# revision 8
# speedup vs baseline: 1.0172x; 1.0172x over previous
"""Multi-head attention (B=2, S=2048, D=1024, H=16) on 8 NeuronCores.

Sharding: core c -> (batch b = c // 4, head-group g = c % 4, 4 heads each).
Each core computes its 4 heads' attention for its batch plus the partial
output projection (ctx_shard @ WO_shard.T).T; the host sums the 4 partials
per batch, adds the bias, and patches fully-masked query rows (where the
reference's softmax degenerates to uniform attention).

Device kernel layout notes:
  - x and the weight shards are pre-transposed on the host and fed as bf16.
  - Q,K are produced in [dk, s] layout (head-pair stacked on partitions) so
    scores come out transposed: S_t[k, q]. The two heads of a pair run as
    concurrent row-group matmuls (K=64 each).
  - Padding mask is applied by zeroing masked rows of V (and of the ones
    column), causal mask by multiplying the 128-col diagonal block of band
    tiles with a precomputed 0/1 triangle.
  - Softmax normalization is deferred past the attention loop: V carries an
    extra ones column so P@V also accumulates row sums L[q]; unnormalized
    ctx and L are staged to SBUF (bf16), L rows bounce through DRAM where a
    reshape to [128, f] makes the reciprocal cheap, and a partition-broadcast
    DMA returns 1/L for the normalize multiplies.
  - Emission order sets scheduler priority: attention(0,0) is emitted right
    after its minimal projection prefix so exp starts early; the remaining
    projections and the per-qt out-projections are emitted later and the
    Tile list scheduler hoists them into the exp-paced PE gaps.
  - A burst of dummy matmuls at t=0 warms the PE HAM clock gate during the
    input-DMA window.
"""

import os
import sys

import numpy as np

sys.path.insert(0, "/opt/trn_rl_repo")
os.environ.setdefault("MYCRO_LOCAL_CACHE", "1")

import ml_dtypes

import concourse.bass as bass
import concourse.tile as tile
from concourse import bacc, mybir
from concourse.bass_utils import run_bass_kernel_spmd

B, S, D, H = 2, 2048, 1024, 16
DK = D // H          # 64
NCORES = 8
HPC = H // (NCORES // B)   # heads per core = 4
DSH = HPC * DK             # 256: per-core shard of the model dim
NKC = S // 128             # 16 key chunks of 128
TRI_W = 384 + 512          # causal strip width
N_WARM = 160               # HAM warmup matmuls

BF = mybir.dt.bfloat16
F32 = mybir.dt.float32
EXP = mybir.ActivationFunctionType.Exp

_NC_CACHE: list = []


def _emit(tc: tile.TileContext, ctx):
    nc = tc.nc

    xT = nc.dram_tensor("xT", [D, S], BF, kind="ExternalInput").ap()
    wqt = nc.dram_tensor("wqt", [D, DSH], BF, kind="ExternalInput").ap()
    wkt = nc.dram_tensor("wkt", [D, DSH], BF, kind="ExternalInput").ap()
    wvt = nc.dram_tensor("wvt", [D, DSH], BF, kind="ExternalInput").ap()
    wot = nc.dram_tensor("wot", [DSH, D], BF, kind="ExternalInput").ap()
    pad0 = nc.dram_tensor("pad0", [128, NKC], F32, kind="ExternalInput").ap()
    tri = nc.dram_tensor("tri", [128, TRI_W], BF, kind="ExternalInput").ap()
    yT = nc.dram_tensor("yT", [D, S], BF, kind="ExternalOutput").ap()

    persist = ctx.enter_context(tc.tile_pool(name="persist", bufs=1))
    sc_pool = ctx.enter_context(tc.tile_pool(name="scps", bufs=2, space="PSUM"))
    ct_pool = ctx.enter_context(tc.tile_pool(name="ctps", bufs=4, space="PSUM"))
    pu_pool = ctx.enter_context(tc.tile_pool(name="pu", bufs=4))
    work = ctx.enter_context(tc.tile_pool(name="work", bufs=4))
    dpool = ctx.enter_context(tc.tile_pool(name="dram", bufs=1, space="DRAM"))

    xs = persist.tile([128, 8, S], BF)
    wq_s = persist.tile([128, 8, DSH], BF)
    wk_s = persist.tile([128, 8, DSH], BF)
    wv_s = persist.tile([128, 8, DSH], BF)
    wo_s = persist.tile([128, 2, D], BF)
    pad_s = persist.tile([128, NKC], F32)
    tri_s = persist.tile([128, TRI_W], BF)
    qt2 = persist.tile([128, 2, S], BF)
    kt2 = persist.tile([128, 2, S], BF)
    vp = persist.tile([128, NKC, 65 * HPC], BF)
    ctn = persist.tile([128, 2, S], BF)
    ctu = persist.tile([65, 16, 512], BF)     # unnormalized ctx + L, per (h, qt)
    wrm = persist.tile([128, 128], BF)        # HAM warmup operand
    rc0 = persist.tile([128, 32], BF)         # pair-0 L reshaped for reciprocal
    rc1 = [persist.tile([128, 8], BF, name=f"rc1_{i}", tag=f"rc1_{i}")
           for i in range(4)]
    ldram = dpool.tile([16, 512], BF)         # raw L rows, r = hp*8 + qt*2 + idx
    ldram2 = dpool.tile([16, 512], BF)        # 1/L rows

    # ---- HAM warmup: keep the PE busy through the input-DMA window ----
    nc.vector.memset(wrm, 0.0)
    wps = ct_pool.tile([64, 64], F32, tag="ct", name="wps")
    for _ in range(N_WARM):
        nc.tensor.matmul(wps, wrm[:, 0:64], wrm[:, 64:128], start=True, stop=True)

    # ---- input DMAs; scalar queue kept clean so EXP dispatch is not delayed
    xr = xT.rearrange("(c p) s -> p c s", p=128)
    wqr = wqt.rearrange("(c p) j -> p c j", p=128)
    wkr = wkt.rearrange("(c p) j -> p c j", p=128)
    wvr = wvt.rearrange("(c p) j -> p c j", p=128)
    nc.sync.dma_start(out=pad_s, in_=pad0)
    nc.sync.dma_start(out=tri_s, in_=tri)
    engs = [nc.sync, nc.gpsimd]
    ei = 0

    def dma_rr(out, in_):
        nonlocal ei
        engs[ei % len(engs)].dma_start(out=out, in_=in_)
        ei += 1

    # pair-0 weight halves + first x quarter first: the attention(0,0)
    # prefix (Q/K st0 + V chunks 0-3) becomes compute-ready ASAP
    for c in range(8):
        dma_rr(wq_s[:, c, 0:128], wqr[:, c, 0:128])
        dma_rr(wk_s[:, c, 0:128], wkr[:, c, 0:128])
        dma_rr(xs[:, c, 0:512], xr[:, c, 0:512])
    for c in range(8):
        dma_rr(wv_s[:, c, :], wvr[:, c, :])
        dma_rr(xs[:, c, 512:1024], xr[:, c, 512:1024])
    for c in range(8):
        dma_rr(wq_s[:, c, 128:256], wqr[:, c, 128:256])
        dma_rr(wk_s[:, c, 128:256], wkr[:, c, 128:256])
        dma_rr(xs[:, c, 1024:2048], xr[:, c, 1024:2048])
    wor = wot.rearrange("(c p) o -> p c o", p=128)
    for c in range(2):
        dma_rr(wo_s[:, c, :], wor[:, c, :])
    nc.vector.memset(vp, 1.0)

    def qk_proj(dht, use_act, sts):
        """Project Q and K s-tiles `sts` for head pair `dht`."""
        for wi, (wsb, dst) in enumerate(((wq_s, qt2), (wk_s, kt2))):
            pss = [
                ct_pool.tile([128, 512], F32, tag="ct", name=f"qkps{si}")
                for si in range(len(sts))
            ]
            for dc in range(8):
                for st, ps in zip(sts, pss):
                    nc.tensor.matmul(
                        ps,
                        wsb[:, dc, 128 * dht : 128 * dht + 128],
                        xs[:, dc, 512 * st : 512 * st + 512],
                        start=(dc == 0),
                        stop=(dc == 7),
                    )
            for st, ps in zip(sts, pss):
                sl = dst[:, dht, 512 * st : 512 * st + 512]
                if use_act:
                    nc.scalar.copy(sl, ps)
                else:
                    nc.vector.tensor_copy(out=sl, in_=ps)

    def v_proj(sc_lo, sc_hi):
        for sc in range(sc_lo, sc_hi):
            ps = ct_pool.tile([128, DSH], F32, tag="ct")
            for dc in range(8):
                nc.tensor.matmul(
                    ps,
                    xs[:, dc, 128 * sc : 128 * sc + 128],
                    wv_s[:, dc, :],
                    start=(dc == 0),
                    stop=(dc == 7),
                )
            vcols = vp[:, sc, :].rearrange("p (h u) -> p h u", u=65)[:, :, 0:64]
            nc.vector.tensor_scalar_mul(
                vcols, ps.rearrange("p (h u) -> p h u", u=64), pad_s[:, sc : sc + 1]
            )
            ones_cols = vp[:, sc, :].rearrange("p (h u) -> p h u", u=65)[:, :, 64:65]
            nc.vector.tensor_scalar_mul(ones_cols, ones_cols, pad_s[:, sc : sc + 1])

    def attention(hp, qt):
        Q0 = 512 * qt
        nkc = 4 * qt + 4
        ct_e = ct_pool.tile([65, 512], F32, tag="ct")
        ct_o = ct_pool.tile([65, 512], F32, tag="ct")
        he, ho = 2 * hp, 2 * hp + 1
        for g in range(0, nkc, 2):
            group = []
            # scores for both kc of the group first: lets exp(kc) overlap
            # scores(kc+1) and batches the K=64 row-group config on the PE
            for kc in (g, g + 1):
                K0 = 128 * kc
                band = K0 >= Q0
                qs = K0 if band else Q0
                w = Q0 + 512 - qs
                co = qs - Q0
                sc = sc_pool.tile([128, 1024], F32, tag="slot")
                nc.tensor.matmul(
                    sc[:, 0:w], kt2[0:64, hp, K0 : K0 + 128],
                    qt2[0:64, hp, qs : qs + w], start=True, stop=True,
                )
                nc.tensor.matmul(
                    sc[:, 512 : 512 + w], kt2[64:128, hp, K0 : K0 + 128],
                    qt2[64:128, hp, qs : qs + w], start=True, stop=True,
                )
                group.append((kc, band, w, co, sc))
            pus = []
            for kc, band, w, co, sc in group:
                pu = pu_pool.tile([128, 1024], BF, tag="pu")
                sc2 = sc.rearrange("p (t f) -> p t f", t=2)[:, :, 0:w]
                pu2 = pu.rearrange("p (t f) -> p t f", t=2)[:, :, 0:w]
                nc.scalar.activation(out=pu2, in_=sc2, func=EXP, scale=0.125)
                if band:
                    # only the leading 128 cols hold the diagonal triangle;
                    # the rest of the band tile is fully live
                    mw = min(128, w)
                    tsl = tri_s[:, 384 : 384 + mw]
                    nc.vector.tensor_mul(pu[:, 0:mw], pu[:, 0:mw], tsl)
                    nc.vector.tensor_mul(
                        pu[:, 512 : 512 + mw], pu[:, 512 : 512 + mw], tsl
                    )
                pus.append(pu)
            for (kc, band, w, co, sc), pu in zip(group, pus):
                nc.tensor.matmul(
                    ct_e[:, co : co + w],
                    vp[:, kc, 65 * he : 65 * he + 65], pu[:, 0:w],
                    start=(kc == 0), stop=(kc == nkc - 1),
                )
                nc.tensor.matmul(
                    ct_o[:, co : co + w],
                    vp[:, kc, 65 * ho : 65 * ho + 65], pu[:, 512 : 512 + w],
                    start=(kc == 0), stop=(kc == nkc - 1),
                )
        for idx, cta in ((0, ct_e), (1, ct_o)):
            hq = (2 * hp + idx) * 4 + qt
            nc.vector.tensor_copy(out=ctu[:, hq, :], in_=cta)
            r = hp * 8 + qt * 2 + idx
            nc.sync.dma_start(out=ldram[r : r + 1, :], in_=ctu[64:65, hq, :])

    def _bcast64(r):
        """1/L row r of ldram2 -> [64, 512] SBUF via partition-broadcast DMA."""
        rlb = work.tile([64, 512], BF, tag="rlb")
        src_row = ldram2[r : r + 1, :]
        bsrc = bass.AP(
            tensor=src_row.tensor, offset=src_row.offset,
            ap=[[0, 64]] + list(src_row.ap[1:]),
        )
        nc.sync.dma_start(out=rlb, in_=bsrc)
        return rlb

    def _recip(dst, lo_r, n_r):
        """1/L for ldram rows [lo_r, lo_r+n_r) via [128, f] reshape."""
        f = n_r * 512 // 128
        src = ldram[lo_r : lo_r + n_r, :].rearrange("r (q j) -> (r q) j", j=f)
        nc.sync.dma_start(out=dst, in_=src)
        nc.vector.tensor_scalar_max(dst, dst, 1e-30)
        with nc.allow_low_precision(reason="1/L in bf16; rel-err budget is 2e-2"):
            nc.vector.reciprocal(dst, dst)
        out = ldram2[lo_r : lo_r + n_r, :].rearrange("r (q j) -> (r q) j", j=f)
        nc.sync.dma_start(out=out, in_=dst)

    def norm_apply(hp, qt, idx):
        Q0 = 512 * qt
        hq = (2 * hp + idx) * 4 + qt
        rlb = _bcast64(hp * 8 + qt * 2 + idx)
        if idx == 0:
            nc.vector.tensor_mul(
                ctn[0:64, hp, Q0 : Q0 + 512], ctu[0:64, hq, :], rlb
            )
        else:
            stg = work.tile([64, 512], BF, tag="stg")
            nc.vector.tensor_mul(stg, ctu[0:64, hq, :], rlb)
            nc.sync.dma_start(out=ctn[64:128, hp, Q0 : Q0 + 512], in_=stg)

    def norm_pair0():
        _recip(rc0, 0, 8)
        for qt in range(4):
            for idx in (0, 1):
                norm_apply(0, qt, idx)

    def norm_qt1(qt):
        _recip(rc1[qt], 8 + 2 * qt, 2)
        for idx in (0, 1):
            norm_apply(1, qt, idx)

    def out_proj(st, tail=False):
        yr = yT.rearrange("(ot p) s -> ot p s", p=128)
        for ot in range(8):
            ps = ct_pool.tile([128, 512], F32, tag="ct")
            for c2 in range(2):
                nc.tensor.matmul(
                    ps,
                    wo_s[:, c2, 128 * ot : 128 * ot + 128],
                    ctn[:, c2, 512 * st : 512 * st + 512],
                    start=(c2 == 0),
                    stop=(c2 == 1),
                )
            ystg = work.tile([128, 512], BF, tag="y")
            if tail and ot % 2 == 0:
                nc.scalar.copy(ystg, ps)
            else:
                nc.vector.tensor_copy(out=ystg, in_=ps)
            nc.gpsimd.dma_start(out=yr[ot, :, 512 * st : 512 * st + 512], in_=ystg)

    # ---- emission order == scheduler priority ----
    qk_proj(0, use_act=True, sts=(0,))
    v_proj(0, 4)
    attention(0, 0)
    qk_proj(0, use_act=True, sts=(1,))
    v_proj(4, 8)
    attention(0, 1)
    qk_proj(0, use_act=False, sts=(2,))
    v_proj(8, 12)
    attention(0, 2)
    qk_proj(0, use_act=False, sts=(3,))
    v_proj(12, 16)
    attention(0, 3)
    qk_proj(1, use_act=False, sts=(0, 1))
    norm_pair0()
    attention(1, 0)
    qk_proj(1, use_act=False, sts=(2, 3))
    norm_qt1(0)
    attention(1, 1)
    out_proj(0)
    norm_qt1(1)
    attention(1, 2)
    out_proj(1)
    norm_qt1(2)
    attention(1, 3)
    out_proj(2)
    norm_qt1(3)
    out_proj(3, tail=True)


def build_nc():
    nc = bacc.Bacc(
        "TRN2",
        target_bir_lowering=False,
        debug=False,
        enable_asserts=False,
        num_devices=NCORES,
    )
    from contextlib import ExitStack

    with tile.TileContext(nc) as tc:
        with ExitStack() as ctx:
            _emit(tc, ctx)
    nc.compile()
    return nc


def _get_nc():
    if not _NC_CACHE:
        _NC_CACHE.append(build_nc())
    return _NC_CACHE[0]


def make_tri() -> np.ndarray:
    p = np.arange(128)[:, None]
    v = np.arange(TRI_W)[None, :]
    return (p <= v - 384).astype(np.float32).astype(ml_dtypes.bfloat16)


def make_in_maps(x, mask, WQ, WK, WV, WO):
    bf = ml_dtypes.bfloat16
    tri = make_tri()
    in_maps = []
    for c in range(NCORES):
        b, g = c // (NCORES // B), c % (NCORES // B)
        sl = slice(DSH * g, DSH * g + DSH)
        in_maps.append(
            {
                "xT": np.ascontiguousarray(x[b].T).astype(bf),
                "wqt": np.ascontiguousarray(WQ[sl, :].T).astype(bf),
                "wkt": np.ascontiguousarray(WK[sl, :].T).astype(bf),
                "wvt": np.ascontiguousarray(WV[sl, :].T).astype(bf),
                "wot": np.ascontiguousarray(WO[:, sl].T).astype(bf),
                "pad0": np.ascontiguousarray(
                    (mask[b] == 0).astype(np.float32).reshape(NKC, 128).T
                ),
                "tri": tri,
            }
        )
    return in_maps


def assemble(results, x, mask, WV, WO, bO) -> np.ndarray:
    y = np.zeros((B, S, D), np.float32)
    for c in range(NCORES):
        y[c // (NCORES // B)] += results[c]["yT"].T
    y += bO[None, None, :]
    # Rows i < first-unmasked-index are fully masked in the reference; its
    # softmax then degenerates to uniform attention over all positions.
    for b in range(B):
        nz = np.nonzero(mask[b] == 0)[0]
        t = int(nz[0]) if nz.size else S
        if t > 0:
            vbar = x[b].mean(axis=0) @ WV.T
            yfix = vbar @ WO.T + bO
            y[b, :t, :] = yfix
    return y


def kernel(x, mask, WQ, WK, WV, WO, bO) -> np.ndarray:
    x = np.asarray(x, np.float32)
    mask = np.asarray(mask, np.int32)
    WQ = np.asarray(WQ, np.float32)
    WK = np.asarray(WK, np.float32)
    WV = np.asarray(WV, np.float32)
    WO = np.asarray(WO, np.float32)
    bO = np.asarray(bO, np.float32)

    nc = _get_nc()
    in_maps = make_in_maps(x, mask, WQ, WK, WV, WO)
    res = run_bass_kernel_spmd(nc, in_maps, list(range(NCORES)))
    return assemble(res.results, x, mask, WV, WO, bO)


# revision 13
# speedup vs baseline: 1.0619x; 1.0440x over previous
"""Multi-head attention (B=2, S=2048, D=1024, H=16) on 8 NeuronCores.

Sharding: core c -> (batch b = c // 4, head-group g = c % 4, 4 heads each).
Each core computes its 4 heads' attention for its batch plus the partial
output projection (ctx_shard @ WO_shard.T).T; the host sums the 4 partials
per batch, adds the bias, and patches fully-masked query rows (where the
reference's softmax degenerates to uniform attention).

Device kernel layout notes:
  - x and the weight shards are pre-transposed on the host and fed as bf16.
  - Q,K are produced in [dk, s] layout (head-pair stacked on partitions) so
    scores come out transposed: S_t[k, q]. The two heads of a pair run as
    concurrent row-group matmuls (K=64 each).
  - Padding mask is applied by zeroing masked rows of V (and of the ones
    column), causal mask by multiplying the 128-col diagonal block of band
    tiles with a precomputed 0/1 triangle.
  - Softmax normalization is deferred past the attention loop: V carries an
    extra ones column so P@V also accumulates row sums L[q]; unnormalized
    ctx and L are staged to SBUF (bf16), L rows bounce through DRAM where a
    reshape to [128, f] makes the reciprocal cheap, and a partition-broadcast
    DMA returns 1/L for the normalize multiplies.
  - Emission order sets scheduler priority: attention(0,0) is emitted right
    after its minimal projection prefix so exp starts early; the remaining
    projections and the per-qt out-projections are emitted later and the
    Tile list scheduler hoists them into the exp-paced PE gaps.
  - A burst of dummy matmuls at t=0 warms the PE HAM clock gate during the
    input-DMA window.
"""

import os
import sys

import numpy as np

sys.path.insert(0, "/opt/trn_rl_repo")
os.environ.setdefault("MYCRO_LOCAL_CACHE", "1")

import ml_dtypes

import concourse.bass as bass
import concourse.tile as tile
from concourse import bacc, mybir
from concourse.bass_utils import run_bass_kernel_spmd

B, S, D, H = 2, 2048, 1024, 16
DK = D // H          # 64
NCORES = 8
HPC = H // (NCORES // B)   # heads per core = 4
DSH = HPC * DK             # 256: per-core shard of the model dim
NKC = S // 128             # 16 key chunks of 128
TRI_W = 384 + 512          # causal strip width
N_WARM = 160               # HAM warmup matmuls

BF = mybir.dt.bfloat16
F32 = mybir.dt.float32
EXP = mybir.ActivationFunctionType.Exp

_NC_CACHE: list = []


def _emit(tc: tile.TileContext, ctx):
    nc = tc.nc

    xT = nc.dram_tensor("xT", [D, S], BF, kind="ExternalInput").ap()
    wqt = nc.dram_tensor("wqt", [D, DSH], BF, kind="ExternalInput").ap()
    wkt = nc.dram_tensor("wkt", [D, DSH], BF, kind="ExternalInput").ap()
    wvt = nc.dram_tensor("wvt", [D, DSH], BF, kind="ExternalInput").ap()
    wot = nc.dram_tensor("wot", [DSH, D], BF, kind="ExternalInput").ap()
    pad0 = nc.dram_tensor("pad0", [128, NKC], F32, kind="ExternalInput").ap()
    tri = nc.dram_tensor("tri", [128, TRI_W], BF, kind="ExternalInput").ap()
    yT = nc.dram_tensor("yT", [D, S], BF, kind="ExternalOutput").ap()

    persist = ctx.enter_context(tc.tile_pool(name="persist", bufs=1))
    sc_pool = ctx.enter_context(tc.tile_pool(name="scps", bufs=2, space="PSUM"))
    ct_pool = ctx.enter_context(tc.tile_pool(name="ctps", bufs=3, space="PSUM"))
    nrm_pool = ctx.enter_context(tc.tile_pool(name="nrmps", bufs=1, space="PSUM"))
    pu_pool = ctx.enter_context(tc.tile_pool(name="pu", bufs=4))
    work = ctx.enter_context(tc.tile_pool(name="work", bufs=4))

    xs = persist.tile([128, 8, S], BF)
    wq_s = persist.tile([128, 8, DSH], BF)
    wk_s = persist.tile([128, 8, DSH], BF)
    wv_s = persist.tile([128, 8, DSH], BF)
    wo_s = persist.tile([128, 2, D], BF)
    pad_s = persist.tile([128, NKC], F32)
    tri_s = persist.tile([128, TRI_W], BF)
    qt2 = persist.tile([128, 2, S], BF)
    kt2 = persist.tile([128, 2, S], BF)
    vp = persist.tile([128, NKC, 65 * HPC], BF)
    ctn = persist.tile([128, 2, S], BF)
    # unnormalized ctx + L per (h, qt); rows 65:96 are zero padding so the
    # L row (64) can be fed to the 32x32 stream transpose in place
    ctu = persist.tile([96, 16, 512], BF)
    wrm = persist.tile([128, 128], BF)        # HAM warmup operand
    ones64 = persist.tile([1, 64], BF)        # lhsT of the 1/L broadcast matmul

    # ---- HAM warmup: keep the PE busy through the input-DMA window ----
    nc.vector.memset(wrm, 0.0)
    nc.vector.memset(ones64, 1.0)
    nc.vector.memset(ctu, 0.0)
    wps = ct_pool.tile([64, 64], F32, tag="ct", name="wps")
    for _ in range(N_WARM):
        nc.tensor.matmul(wps, wrm[:, 0:64], wrm[:, 64:128], start=True, stop=True)

    # ---- input DMAs; scalar queue kept clean so EXP dispatch is not delayed
    xr = xT.rearrange("(c p) s -> p c s", p=128)
    wqr = wqt.rearrange("(c p) j -> p c j", p=128)
    wkr = wkt.rearrange("(c p) j -> p c j", p=128)
    wvr = wvt.rearrange("(c p) j -> p c j", p=128)
    nc.sync.dma_start(out=pad_s, in_=pad0)
    nc.sync.dma_start(out=tri_s, in_=tri)
    engs = [nc.sync, nc.gpsimd]
    ei = 0

    def dma_rr(out, in_):
        nonlocal ei
        engs[ei % len(engs)].dma_start(out=out, in_=in_)
        ei += 1

    # pair-0 weight halves + first x quarter first: the attention(0,0)
    # prefix (Q/K st0 + V chunks 0-3) becomes compute-ready ASAP
    for c in range(8):
        dma_rr(wq_s[:, c, 0:128], wqr[:, c, 0:128])
        dma_rr(wk_s[:, c, 0:128], wkr[:, c, 0:128])
        dma_rr(xs[:, c, 0:512], xr[:, c, 0:512])
    for c in range(8):
        dma_rr(wv_s[:, c, :], wvr[:, c, :])
        dma_rr(xs[:, c, 512:1024], xr[:, c, 512:1024])
    for c in range(8):
        dma_rr(wq_s[:, c, 128:256], wqr[:, c, 128:256])
        dma_rr(wk_s[:, c, 128:256], wkr[:, c, 128:256])
        dma_rr(xs[:, c, 1024:2048], xr[:, c, 1024:2048])
    wor = wot.rearrange("(c p) o -> p c o", p=128)
    for c in range(2):
        dma_rr(wo_s[:, c, :], wor[:, c, :])
    nc.vector.memset(vp, 1.0)

    def qk_proj(dht, use_act, sts):
        """Project Q and K s-tiles `sts` for head pair `dht`."""
        for wi, (wsb, dst) in enumerate(((wq_s, qt2), (wk_s, kt2))):
            pss = [
                ct_pool.tile([128, 512], F32, tag="ct", name=f"qkps{si}")
                for si in range(len(sts))
            ]
            for dc in range(8):
                for st, ps in zip(sts, pss):
                    nc.tensor.matmul(
                        ps,
                        wsb[:, dc, 128 * dht : 128 * dht + 128],
                        xs[:, dc, 512 * st : 512 * st + 512],
                        start=(dc == 0),
                        stop=(dc == 7),
                    )
            for st, ps in zip(sts, pss):
                sl = dst[:, dht, 512 * st : 512 * st + 512]
                if use_act:
                    nc.scalar.copy(sl, ps)
                else:
                    nc.vector.tensor_copy(out=sl, in_=ps)

    def v_proj(sc_lo, sc_hi):
        for sc in range(sc_lo, sc_hi):
            ps = ct_pool.tile([128, DSH], F32, tag="ct")
            for dc in range(8):
                nc.tensor.matmul(
                    ps,
                    xs[:, dc, 128 * sc : 128 * sc + 128],
                    wv_s[:, dc, :],
                    start=(dc == 0),
                    stop=(dc == 7),
                )
            vcols = vp[:, sc, :].rearrange("p (h u) -> p h u", u=65)[:, :, 0:64]
            nc.vector.tensor_scalar_mul(
                vcols, ps.rearrange("p (h u) -> p h u", u=64), pad_s[:, sc : sc + 1]
            )
            ones_cols = vp[:, sc, :].rearrange("p (h u) -> p h u", u=65)[:, :, 64:65]
            nc.vector.tensor_scalar_mul(ones_cols, ones_cols, pad_s[:, sc : sc + 1])

    def attention(hp, qt):
        Q0 = 512 * qt
        nkc = 4 * qt + 4
        ct_e = ct_pool.tile([65, 512], F32, tag="ct")
        ct_o = ct_pool.tile([65, 512], F32, tag="ct")
        he, ho = 2 * hp, 2 * hp + 1
        for g in range(0, nkc, 2):
            group = []
            # scores for both kc of the group first: lets exp(kc) overlap
            # scores(kc+1) and batches the K=64 row-group config on the PE
            for kc in (g, g + 1):
                K0 = 128 * kc
                band = K0 >= Q0
                qs = K0 if band else Q0
                w = Q0 + 512 - qs
                co = qs - Q0
                sc = sc_pool.tile([128, 1024], F32, tag="slot")
                nc.tensor.matmul(
                    sc[:, 0:w], kt2[0:64, hp, K0 : K0 + 128],
                    qt2[0:64, hp, qs : qs + w], start=True, stop=True,
                )
                nc.tensor.matmul(
                    sc[:, 512 : 512 + w], kt2[64:128, hp, K0 : K0 + 128],
                    qt2[64:128, hp, qs : qs + w], start=True, stop=True,
                )
                group.append((kc, band, w, co, sc))
            pus = []
            for kc, band, w, co, sc in group:
                pu = pu_pool.tile([128, 1024], BF, tag="pu")
                sc2 = sc.rearrange("p (t f) -> p t f", t=2)[:, :, 0:w]
                pu2 = pu.rearrange("p (t f) -> p t f", t=2)[:, :, 0:w]
                nc.scalar.activation(out=pu2, in_=sc2, func=EXP, scale=0.125)
                if band:
                    # only the leading 128 cols hold the diagonal triangle;
                    # the rest of the band tile is fully live
                    mw = min(128, w)
                    tsl = tri_s[:, 384 : 384 + mw]
                    nc.vector.tensor_mul(pu[:, 0:mw], pu[:, 0:mw], tsl)
                    nc.vector.tensor_mul(
                        pu[:, 512 : 512 + mw], pu[:, 512 : 512 + mw], tsl
                    )
                pus.append(pu)
            for (kc, band, w, co, sc), pu in zip(group, pus):
                nc.tensor.matmul(
                    ct_e[:, co : co + w],
                    vp[:, kc, 65 * he : 65 * he + 65], pu[:, 0:w],
                    start=(kc == 0), stop=(kc == nkc - 1),
                )
                nc.tensor.matmul(
                    ct_o[:, co : co + w],
                    vp[:, kc, 65 * ho : 65 * ho + 65], pu[:, 512 : 512 + w],
                    start=(kc == 0), stop=(kc == nkc - 1),
                )
        for idx, cta in ((0, ct_e), (1, ct_o)):
            hq = (2 * hp + idx) * 4 + qt
            nc.vector.tensor_copy(out=ctu[0:65, hq, :], in_=cta)

    def norm_qt(hp, qt):
        """Normalize ctx of both heads of (hp, qt), DMA-free.

        The L rows (ctu partition 64, zero-padded to 96) stream-transpose so
        each 32-col block's L lands on its own partition; 1/max(L,eps) is a
        cheap FD-32 reciprocal on the stride-32 view; transposing back yields
        a 1/L row that a K=1 matmul broadcasts across 64 PSUM partitions.
        """
        Q0 = 512 * qt
        # [32, 2(idx), 512] view of the L rows of heads 2hp, 2hp+1
        lsrc = ctu[64:96, :, :].rearrange(
            "p (a q) f -> p q a f", a=4)[:, qt, 2 * hp : 2 * hp + 2, :]
        ltr = work.tile([32, 1024], BF, tag="ltr")
        nc.vector.transpose(out=ltr, in_=lsrc)
        lv = ltr.rearrange("p (i j k) -> p i j k", i=2, k=32)[:, :, :, 0:1]
        nc.vector.tensor_scalar_max(lv, lv, 1e-30)
        with nc.allow_low_precision(reason="1/L in bf16; rel-err budget is 2e-2"):
            nc.vector.reciprocal(lv, lv)
        lrow = work.tile([32, 1024], BF, tag="lrow")
        nc.vector.transpose(out=lrow, in_=ltr)
        for idx in (0, 1):
            hq = (2 * hp + idx) * 4 + qt
            rlb = nrm_pool.tile([64, 512], F32, tag="nrm")
            nc.tensor.matmul(
                rlb, ones64, lrow[0:1, 512 * idx : 512 * idx + 512],
                start=True, stop=True,
            )
            if idx == 0:
                nc.vector.tensor_mul(
                    ctn[0:64, hp, Q0 : Q0 + 512], ctu[0:64, hq, :], rlb
                )
            else:
                stg = work.tile([64, 512], BF, tag="stg")
                nc.vector.tensor_mul(stg, ctu[0:64, hq, :], rlb)
                nc.sync.dma_start(out=ctn[64:128, hp, Q0 : Q0 + 512], in_=stg)

    def norm_pair0():
        for qt in range(4):
            norm_qt(0, qt)

    def norm_qt1(qt):
        norm_qt(1, qt)

    def out_proj(st, tail=False):
        yr = yT.rearrange("(ot p) s -> ot p s", p=128)
        for ot in range(8):
            ps = ct_pool.tile([128, 512], F32, tag="ct")
            for c2 in range(2):
                nc.tensor.matmul(
                    ps,
                    wo_s[:, c2, 128 * ot : 128 * ot + 128],
                    ctn[:, c2, 512 * st : 512 * st + 512],
                    start=(c2 == 0),
                    stop=(c2 == 1),
                )
            ystg = work.tile([128, 512], BF, tag="y")
            if tail and ot % 2 == 0:
                nc.scalar.copy(ystg, ps)
            else:
                nc.vector.tensor_copy(out=ystg, in_=ps)
            nc.gpsimd.dma_start(out=yr[ot, :, 512 * st : 512 * st + 512], in_=ystg)

    # ---- emission order == scheduler priority ----
    qk_proj(0, use_act=True, sts=(0,))
    v_proj(0, 4)
    attention(0, 0)
    qk_proj(0, use_act=True, sts=(1,))
    v_proj(4, 8)
    attention(0, 1)
    qk_proj(0, use_act=False, sts=(2,))
    v_proj(8, 12)
    attention(0, 2)
    qk_proj(0, use_act=False, sts=(3,))
    v_proj(12, 16)
    attention(0, 3)
    qk_proj(1, use_act=False, sts=(0,))
    qk_proj(1, use_act=False, sts=(1,))
    norm_pair0()
    attention(1, 0)
    qk_proj(1, use_act=False, sts=(2,))
    qk_proj(1, use_act=False, sts=(3,))
    norm_qt1(0)
    attention(1, 1)
    out_proj(0)
    norm_qt1(1)
    attention(1, 2)
    out_proj(1)
    norm_qt1(2)
    attention(1, 3)
    out_proj(2)
    norm_qt1(3)
    out_proj(3, tail=True)


def build_nc():
    nc = bacc.Bacc(
        "TRN2",
        target_bir_lowering=False,
        debug=False,
        enable_asserts=False,
        num_devices=NCORES,
    )
    from contextlib import ExitStack

    with tile.TileContext(nc) as tc:
        with ExitStack() as ctx:
            _emit(tc, ctx)
    nc.compile()
    return nc


def _get_nc():
    if not _NC_CACHE:
        _NC_CACHE.append(build_nc())
    return _NC_CACHE[0]


def make_tri() -> np.ndarray:
    p = np.arange(128)[:, None]
    v = np.arange(TRI_W)[None, :]
    return (p <= v - 384).astype(np.float32).astype(ml_dtypes.bfloat16)


def make_in_maps(x, mask, WQ, WK, WV, WO):
    bf = ml_dtypes.bfloat16
    tri = make_tri()
    in_maps = []
    for c in range(NCORES):
        b, g = c // (NCORES // B), c % (NCORES // B)
        sl = slice(DSH * g, DSH * g + DSH)
        in_maps.append(
            {
                "xT": np.ascontiguousarray(x[b].T).astype(bf),
                "wqt": np.ascontiguousarray(WQ[sl, :].T).astype(bf),
                "wkt": np.ascontiguousarray(WK[sl, :].T).astype(bf),
                "wvt": np.ascontiguousarray(WV[sl, :].T).astype(bf),
                "wot": np.ascontiguousarray(WO[:, sl].T).astype(bf),
                "pad0": np.ascontiguousarray(
                    (mask[b] == 0).astype(np.float32).reshape(NKC, 128).T
                ),
                "tri": tri,
            }
        )
    return in_maps


def assemble(results, x, mask, WV, WO, bO) -> np.ndarray:
    y = np.zeros((B, S, D), np.float32)
    for c in range(NCORES):
        y[c // (NCORES // B)] += results[c]["yT"].T
    y += bO[None, None, :]
    # Rows i < first-unmasked-index are fully masked in the reference; its
    # softmax then degenerates to uniform attention over all positions.
    for b in range(B):
        nz = np.nonzero(mask[b] == 0)[0]
        t = int(nz[0]) if nz.size else S
        if t > 0:
            vbar = x[b].mean(axis=0) @ WV.T
            yfix = vbar @ WO.T + bO
            y[b, :t, :] = yfix
    return y


def kernel(x, mask, WQ, WK, WV, WO, bO) -> np.ndarray:
    x = np.asarray(x, np.float32)
    mask = np.asarray(mask, np.int32)
    WQ = np.asarray(WQ, np.float32)
    WK = np.asarray(WK, np.float32)
    WV = np.asarray(WV, np.float32)
    WO = np.asarray(WO, np.float32)
    bO = np.asarray(bO, np.float32)

    nc = _get_nc()
    in_maps = make_in_maps(x, mask, WQ, WK, WV, WO)
    res = run_bass_kernel_spmd(nc, in_maps, list(range(NCORES)))
    return assemble(res.results, x, mask, WV, WO, bO)
